# revision 1
# baseline (speedup 1.0000x reference)
"""Trainium2 Bass kernel for nn_DMRI2INetworkLayer (additive-attention pooling).

Reference (per batch row b):
    pre[s,h]  = X_item[b]@Wc + X_series[b,s]@We + pos[s]@Wp
    scores[s] = sum_h z[h]*tanh(pre[s,h])
    score_sum = sum_s where(mask, scores, 0)
    attn      = softmax(where(mask, scores, -inf))
    out[b]    = concat(sum_s attn[s]*X_series[b,s], score_sum)

Sharding: data-parallel over batch B=4096 across 8 NeuronCores (512 rows
per core). Host does layout/dtype marshalling only; all math on device.

Per-core device design (heavy operands bf16, f32 PSUM accumulation):
  - xt [128d, bc*200s] (b-major/s-minor cols) streams as rhs of the main
    matmul; lhsT=[We|We] in two PE col-groups -> pre-PSUM [(h x 2str), 400].
  - pos/item bias terms fold in as two more full-K accumulating matmuls:
    lhsT = [I64;0] / [0;I64] (zero-padded identities), rhs = pT2 (static)
    and cT via a step-0 broadcast AP. All matmuls keep K=128 (partial-row
    LDWEIGHTS compositions hang the HW).
  - tanh on ACT (PSUM -> SBUF bf16); z-dot via lhsT=[z;0|0;z] (M=32,
    zero-padded) matmuls col-packed 4x into one scores-PSUM tile.
  - scores: DVE drain -> 8 [2,200] reshape DMAs -> dense [bt,200] tiles
    (rows carry a fixed 16-row permutation); masked softmax on DVE/ACT.
  - weighted sum over s: per-(b, s-chunk) fused bf16 matmuls with the
    X-natural slice (K=128, s>=200 zero-padded) stationary and the attn^T
    column as rhs, accumulating into one PSUM bank [128d, bc] via
    per-element has_written semantics.
  - output: PE-transpose [d,b]->[b,d]; score_sum un-permuted on host.
"""
import os
import sys

sys.path.insert(0, "/opt/trn_rl_repo")

import numpy as np
import ml_dtypes
from contextlib import ExitStack

import concourse.bass as bass
import concourse.bacc as bacc
import concourse.tile as tile
from concourse import mybir
from concourse.bass_utils import run_bass_kernel_spmd

BF = mybir.dt.bfloat16
F32 = mybir.dt.float32
BF_NP = ml_dtypes.bfloat16

N_CORES = 8
B, S, D, H = 4096, 200, 128, 64
BC = B // N_CORES          # batch rows per core
SP = 256                   # padded S (two 128-row chunks; s>=200 zeroed)
GB = 4                     # b's per matmul group (2 col-group pairs)
TB = 16                    # b's per scores-PSUM tile (4 zz col-groups)

STAGE = os.environ.get("K_STAGE", "full")  # scores | softmax | attnT | full
REPEAT = int(os.environ.get("K_REPEAT", "1"))  # timing: repeat whole pipeline

# dense-scores row i within a 16-b block holds b16 = PERM16[i]
PERM16 = np.array([4 * (i // 4) + 2 * (i % 2) + ((i % 4) // 2) for i in range(16)])
PERM16_INV = np.argsort(PERM16)


def perm_full(bc):
    j = np.arange(bc)
    return (j // 16) * 16 + PERM16[j % 16]


_CACHE = {}


def build_nc(bc=BC):
    key = (bc, STAGE, REPEAT)
    if key in _CACHE:
        return _CACHE[key]
    bt_sz = min(128, bc)
    n_bt = bc // bt_sz
    n_tbt = bt_sz // TB        # 16-b tiles per softmax tile

    nc = bacc.Bacc("TRN2", target_bir_lowering=False, num_devices=N_CORES)

    xt = nc.declare_dram_parameter("xt", [D, bc * S], BF, isOutput=False)
    xn0 = nc.declare_dram_parameter("xn0", [128, bc * D], BF, isOutput=False)
    xn1 = nc.declare_dram_parameter("xn1", [128, bc * D], BF, isOutput=False)
    wew = nc.declare_dram_parameter("wew", [D, 128], BF, isOutput=False)
    ii = nc.declare_dram_parameter("ii", [128, 128], BF, isOutput=False)
    zz = nc.declare_dram_parameter("zz", [128, 32], BF, isOutput=False)
    ptab = nc.declare_dram_parameter("ptab", [D, S], BF, isOutput=False)
    wpw = nc.declare_dram_parameter("wpw", [D, 128], BF, isOutput=False)
    xitT = nc.declare_dram_parameter("xitT", [D, bc], BF, isOutput=False)
    wcw = nc.declare_dram_parameter("wcw", [D, 128], BF, isOutput=False)
    idbf = nc.declare_dram_parameter("idbf", [128, 128], BF, isOutput=False)
    idf = nc.declare_dram_parameter("idf", [128, 128], F32, isOutput=False)
    mbp = nc.declare_dram_parameter("mbp", [bc, S], F32, isOutput=False)
    m01p = nc.declare_dram_parameter("m01p", [bc, S], F32, isOutput=False)
    out_attn = nc.declare_dram_parameter("out_attn", [bc, D], F32, isOutput=True)
    out_ssum = nc.declare_dram_parameter("out_ssum", [bc, 1], F32, isOutput=True)

    with tile.TileContext(nc) as tc, ExitStack() as ctx:
        const = ctx.enter_context(tc.tile_pool(name="const", bufs=1))
        xtp = ctx.enter_context(tc.tile_pool(name="xtp", bufs=3))
        xnp = ctx.enter_context(tc.tile_pool(name="xnp", bufs=n_tbt + 2))
        thp = ctx.enter_context(tc.tile_pool(name="thp", bufs=6))
        scp = ctx.enter_context(tc.tile_pool(name="scp", bufs=3))
        smp = ctx.enter_context(tc.tile_pool(name="smp", bufs=2))
        atp = ctx.enter_context(tc.tile_pool(name="atp", bufs=2))
        outp = ctx.enter_context(tc.tile_pool(name="outp", bufs=2))
        pre_ps = ctx.enter_context(tc.tile_pool(name="pre_ps", bufs=2, space="PSUM"))
        sc_ps = ctx.enter_context(tc.tile_pool(name="sc_ps", bufs=2, space="PSUM"))
        o5_ps = ctx.enter_context(tc.tile_pool(name="o5_ps", bufs=1, space="PSUM"))
        t_ps = ctx.enter_context(tc.tile_pool(name="t_ps", bufs=1, space="PSUM"))

        # ---------- constants ----------
        def cdma(shape, dt_, src, tag):
            t = const.tile(shape, dt_, tag=tag)
            nc.sync.dma_start(t[:], src)
            return t

        wew_t = cdma([D, 128], BF, wew[:], "wew_t")
        ii_t = cdma([128, 128], BF, ii[:], "ii_t")   # [:,0:64]=[I64;0], [:,64:]=[0;I64]
        zz_t = cdma([128, 32], BF, zz[:], "zz_t")
        pos_t = cdma([D, S], BF, ptab[:], "pos_t")
        wpw_t = cdma([D, 128], BF, wpw[:], "wpw_t")
        xitT_t = cdma([D, bc], BF, xitT[:], "xitT_t")
        wcw_t = cdma([D, 128], BF, wcw[:], "wcw_t")
        idbf_t = cdma([128, 128], BF, idbf[:], "idbf_t")
        idf_t = cdma([128, 128], F32, idf[:], "idf_t")

        # ---------- phase 0: pT2 (rows 0-63, rows 64-127 zero) ----------
        ph_ps = t_ps.tile([128, 512], F32, tag="tps")
        nc.tensor.matmul(ph_ps[0:64, 0:S], wpw_t[:, 0:64], pos_t[:],
                         start=True, stop=True, tile_position=(0, 0),
                         skip_group_check=True)
        pt2 = const.tile([128, 2 * S], BF, tag="pt2")
        nc.vector.memset(pt2[:], 0.0)
        nc.vector.tensor_copy(pt2[0:64, 0:S], ph_ps[0:64, 0:S])
        nc.vector.tensor_copy(pt2[0:64, S:2 * S], ph_ps[0:64, 0:S])

        # cT at rows 64-127 (rows 0-63 zero)
        ct = const.tile([128, bc], BF, tag="ct")
        nc.vector.memset(ct[:], 0.0)
        for j in range((bc + 511) // 512):
            n = min(512, bc - j * 512)
            c_ps = t_ps.tile([128, 512], F32, tag="tps")
            nc.tensor.matmul(c_ps[64:128, 0:n], wcw_t[:, 64:128],
                             xitT_t[:, j * 512:j * 512 + n],
                             start=True, stop=True, tile_position=(0, 64),
                             skip_group_check=True)
            nc.vector.tensor_copy(ct[64:128, j * 512:j * 512 + n], c_ps[64:128, 0:n])

        # ---------- step5 accumulator ----------
        o5 = o5_ps.tile([D, bc], F32)
        nc.vector.memset(o5[:], 0.0)
        step5_n = 0

        for bt_rep in range(n_bt * REPEAT):
            bt = bt_rep % n_bt
            sc_dense = smp.tile([bt_sz, S], F32, tag="sc_dense")
            xn_tiles = []
            for tb_i in range(n_tbt):
                tb = bt * n_tbt + tb_i
                xt_t = xtp.tile([D, TB * S], BF, tag="xt_t")
                nc.sync.dma_start(xt_t[:], xt[:, tb * TB * S:(tb + 1) * TB * S])
                xn0_t = xnp.tile([128, TB * D], BF, tag="xn0_t")
                nc.sync.dma_start(xn0_t[:], xn0[:, tb * TB * D:(tb + 1) * TB * D])
                xn1_t = xnp.tile([128, TB * D], BF, tag="xn1_t")
                nc.sync.dma_start(xn1_t[:], xn1[:, tb * TB * D:(tb + 1) * TB * D])
                xn_tiles.append((xn0_t, xn1_t))

                sc_psum = sc_ps.tile([128, 2 * S], F32, tag="sc_psum")
                for g in range(TB // GB):
                    pre = pre_ps.tile([128, 2 * S], F32, tag="pre")
                    c0 = g * GB * S
                    b0 = tb * TB + g * GB
                    ctA = ct[:, b0:b0 + 2, None].broadcast_to((128, 2, S))
                    ctB = ct[:, b0 + 2:b0 + 4, None].broadcast_to((128, 2, S))
                    # rows 0-63 of pre: b0, b0+1
                    nc.tensor.matmul(pre[0:64, :], wew_t[:, 0:64],
                                     xt_t[:, c0:c0 + 2 * S],
                                     start=True, stop=False, tile_position=(0, 0),
                                     skip_group_check=True)
                    nc.tensor.matmul(pre[0:64, :], ii_t[:, 0:64], pt2[:],
                                     start=False, stop=False, tile_position=(0, 0),
                                     skip_group_check=True)
                    nc.tensor.matmul(pre[0:64, :], ii_t[:, 64:128], ctA,
                                     start=False, stop=True, tile_position=(0, 0),
                                     skip_group_check=True)
                    # rows 64-127 of pre: b0+2, b0+3
                    nc.tensor.matmul(pre[64:128, :], wew_t[:, 64:128],
                                     xt_t[:, c0 + 2 * S:c0 + 4 * S],
                                     start=True, stop=False, tile_position=(0, 64),
                                     skip_group_check=True)
                    nc.tensor.matmul(pre[64:128, :], ii_t[:, 0:64], pt2[:],
                                     start=False, stop=False, tile_position=(0, 64),
                                     skip_group_check=True)
                    nc.tensor.matmul(pre[64:128, :], ii_t[:, 64:128], ctB,
                                     start=False, stop=True, tile_position=(0, 64),
                                     skip_group_check=True)
                    th = thp.tile([128, 2 * S], BF, tag="th")
                    nc.scalar.activation(th[:], pre[:],
                                         mybir.ActivationFunctionType.Tanh)
                    nc.tensor.matmul(sc_psum[32 * g:32 * g + 32, :], zz_t[:], th[:],
                                     start=True, stop=True,
                                     tile_position=(0, 32 * g),
                                     skip_group_check=True)
                sc_sp = scp.tile([128, 2 * S], F32, tag="sc_sp")
                nc.vector.tensor_copy(sc_sp[:], sc_psum[:])
                # scatter row-pairs {32g,32g+1} x (b01,s) -> dense rows 4g+2h+r
                r0 = tb_i * TB
                for g in range(4):
                    for h in range(2):
                        src = sc_sp[32 * g:32 * g + 2, h * S:(h + 1) * S]
                        dr = r0 + 4 * g + 2 * h
                        nc.sync.dma_start(sc_dense[dr:dr + 2, :], src)

            # ---------- masked softmax ----------
            if STAGE == "scores":
                nc.sync.dma_start(out_attn[bt * bt_sz:(bt + 1) * bt_sz, :],
                                  sc_dense[:, 0:D])
                zs = smp.tile([bt_sz, 1], F32, tag="zs")
                nc.vector.memset(zs[:], 0.0)
                nc.sync.dma_start(out_ssum[bt * bt_sz:(bt + 1) * bt_sz, :], zs[:])
                continue
            mb_t = smp.tile([bt_sz, S], F32, tag="mb_t")
            nc.sync.dma_start(mb_t[:], mbp[bt * bt_sz:(bt + 1) * bt_sz, :])
            m01_t = smp.tile([bt_sz, S], F32, tag="m01_t")
            nc.sync.dma_start(m01_t[:], m01p[bt * bt_sz:(bt + 1) * bt_sz, :])

            sc_m = smp.tile([bt_sz, S], F32, tag="sc_m")
            ssum = smp.tile([bt_sz, 1], F32, tag="ssum")
            nc.vector.tensor_mul(sc_m[:], sc_dense[:], m01_t[:])
            nc.vector.reduce_sum(ssum[:], sc_m[:], axis=mybir.AxisListType.X)
            sc_soft = smp.tile([bt_sz, S], F32, tag="sc_soft")
            nc.vector.tensor_add(sc_soft[:], sc_m[:], mb_t[:])
            nmax = smp.tile([bt_sz, 1], F32, tag="nmax")
            nc.vector.tensor_reduce(nmax[:], sc_soft[:], axis=mybir.AxisListType.X,
                                    op=mybir.AluOpType.max, negate=True)
            expd = smp.tile([bt_sz, S], F32, tag="expd")
            nc.scalar.activation(expd[:], sc_soft[:],
                                 mybir.ActivationFunctionType.Exp,
                                 bias=nmax[:], scale=1.0)
            den = smp.tile([bt_sz, 1], F32, tag="den")
            nc.vector.reduce_sum(den[:], expd[:], axis=mybir.AxisListType.X)
            rden = smp.tile([bt_sz, 1], F32, tag="rden")
            nc.vector.reciprocal(rden[:], den[:])
            attn = atp.tile([bt_sz, SP], BF, tag="attn")
            nc.vector.memset(attn[:], 0.0)
            nc.vector.tensor_scalar_mul(attn[:, 0:S], expd[:], rden[:])
            nc.sync.dma_start(out_ssum[bt * bt_sz:(bt + 1) * bt_sz, :], ssum[:])

            if STAGE == "softmax":
                nc.sync.dma_start(out_attn[bt * bt_sz:(bt + 1) * bt_sz, :],
                                  expd[:, 0:D])
                continue

            # ---------- attn^T (two full 128-col blocks) ----------
            at_ps = t_ps.tile([128, 128], BF, tag="tps_bf")
            nc.tensor.transpose(at_ps[0:128, 0:bt_sz], attn[:, 0:128],
                                idbf_t[0:bt_sz, 0:bt_sz])
            atT_lo = atp.tile([128, bt_sz], BF, tag="atT_lo")
            nc.vector.tensor_copy(atT_lo[:], at_ps[0:128, 0:bt_sz])
            at_ps2 = t_ps.tile([128, 128], BF, tag="tps_bf")
            nc.tensor.transpose(at_ps2[0:128, 0:bt_sz], attn[:, 128:256],
                                idbf_t[0:bt_sz, 0:bt_sz])
            atT_hi = atp.tile([128, bt_sz], BF, tag="atT_hi")
            nc.vector.tensor_copy(atT_hi[:], at_ps2[0:128, 0:bt_sz])

            # ---------- weighted sum over s ----------
            if STAGE == "attnT":
                ats = outp.tile([128, bt_sz], F32, tag="ats")
                nc.vector.tensor_copy(ats[:], atT_lo[:])
                nc.sync.dma_start(out_attn[bt * bt_sz:(bt + 1) * bt_sz, :],
                                  ats[0:bt_sz, 0:D])
                continue
            for tb_i in range(n_tbt):
                xn0_t, xn1_t = xn_tiles[tb_i]
                for bi in range(TB):
                    b_in_bt = tb_i * TB + bi
                    b_loc = bt * bt_sz + b_in_bt
                    j = (b_in_bt // 16) * 16 + int(PERM16_INV[b_in_bt % 16])
                    step5_n += 2
                    nc.tensor.matmul(o5[:, b_loc:b_loc + 1],
                                     xn0_t[:, bi * D:(bi + 1) * D],
                                     atT_lo[:, j:j + 1],
                                     start=False, stop=False,
                                     skip_group_check=True)
                    nc.tensor.matmul(o5[:, b_loc:b_loc + 1],
                                     xn1_t[:, bi * D:(bi + 1) * D],
                                     atT_hi[:, j:j + 1],
                                     start=False, stop=(step5_n == 2 * bc * REPEAT),
                                     skip_group_check=True)

        # ---------- drain weighted sum, transpose to [b, d], store ----------
        if STAGE == "full":
            o5_s = outp.tile([D, bc], F32, tag="o5_s")
            nc.vector.tensor_copy(o5_s[:], o5[:])
            for t in range((bc + 127) // 128):
                n = min(128, bc - t * 128)
                ot_ps = t_ps.tile([128, 128], F32, tag="otps")
                nc.tensor.transpose(ot_ps[0:n, :], o5_s[:, t * 128:t * 128 + n],
                                    idf_t[:])
                ob = outp.tile([128, D], F32, tag="ob")
                nc.vector.tensor_copy(ob[0:n, :], ot_ps[0:n, :])
                nc.sync.dma_start(out_attn[t * 128:t * 128 + n, :], ob[0:n, :])
        else:
            o5_d = outp.tile([D, bc], F32, tag="o5_s")
            nc.vector.tensor_copy(o5_d[:], o5[:])

    nc.compile()
    _CACHE[key] = nc
    return nc


def _prep_core(Xs, Xit, pos, mask, We, Wp, Wc, z, bc):
    """Host-side marshalling (layout/dtype only) for one core's shard."""
    d = {}
    d["xt"] = np.ascontiguousarray(Xs.transpose(2, 0, 1).reshape(D, bc * S)).astype(BF_NP)
    xn = Xs.transpose(1, 0, 2)                     # [S, bc, D]
    d["xn0"] = np.ascontiguousarray(xn[0:128].reshape(128, bc * D)).astype(BF_NP)
    xn1 = np.zeros((128, bc, D), np.float32)
    xn1[0:S - 128] = xn[128:S]
    d["xn1"] = xn1.reshape(128, bc * D).astype(BF_NP)
    d["wew"] = np.concatenate([We, We], 1).astype(BF_NP)
    i64 = np.eye(64, dtype=np.float32)
    iim = np.zeros((128, 128), np.float32)
    iim[0:64, 0:64] = i64          # [I64; 0] for the pos fold
    iim[64:128, 64:128] = i64      # [0; I64] for the item fold
    d["ii"] = iim.astype(BF_NP)
    zzm = np.zeros((128, 32), np.float32)
    zzm[0:64, 0] = z
    zzm[64:128, 1] = z
    d["zz"] = zzm.astype(BF_NP)
    d["ptab"] = np.ascontiguousarray(pos.T).astype(BF_NP)
    d["wpw"] = np.concatenate([Wp, Wp], 1).astype(BF_NP)
    d["xitT"] = np.ascontiguousarray(Xit.T).astype(BF_NP)
    d["wcw"] = np.concatenate([Wc, Wc], 1).astype(BF_NP)
    d["idbf"] = np.eye(128, dtype=np.float32).astype(BF_NP)
    d["idf"] = np.eye(128, dtype=np.float32)
    pf = perm_full(bc)
    m01 = mask.astype(np.float32)
    d["m01p"] = np.ascontiguousarray(m01[pf])
    d["mbp"] = np.ascontiguousarray((m01[pf] - 1.0) * 1.0e30)
    return d


def _unshard(results, bc):
    pf = perm_full(bc)
    outs = []
    for k in range(len(results)):
        attn_out = results[k]["out_attn"]
        ssum_perm = results[k]["out_ssum"]
        ssum = np.empty_like(ssum_perm)
        ssum[pf] = ssum_perm
        outs.append(np.concatenate([attn_out, ssum], axis=1))
    return np.concatenate(outs, axis=0)


def make_in_maps(X_series, pos_series, X_item, valid_mask, Wc, Wp, We, z, bc):
    in_maps = []
    for k in range(N_CORES):
        sl = slice(k * bc, (k + 1) * bc)
        in_maps.append(_prep_core(np.asarray(X_series[sl], np.float32),
                                  np.asarray(X_item[sl], np.float32),
                                  np.asarray(pos_series, np.float32),
                                  np.asarray(valid_mask[sl]),
                                  np.asarray(We, np.float32),
                                  np.asarray(Wp, np.float32),
                                  np.asarray(Wc, np.float32),
                                  np.asarray(z, np.float32), bc))
    return in_maps


def kernel(X_series, pos_series, X_item, valid_mask, Wc, Wp, We, z):
    X_series = np.asarray(X_series, np.float32)
    bc = X_series.shape[0] // N_CORES
    nc = build_nc(bc)
    in_maps = make_in_maps(X_series, pos_series, X_item, valid_mask,
                           Wc, Wp, We, z, bc)
    res = run_bass_kernel_spmd(nc, in_maps, list(range(N_CORES)))
    return _unshard(res.results, bc)

